# revision 1
# baseline (speedup 1.0000x reference)
# ARFSA attention kernel for 8 TRN2 NeuronCores (Bass/Tile), v4.
#
# Reference computation (per batch b, channel c):
#   q = Wq x + bq ; k = Wk x + bk ; v = Wv x + bv          (1x1 convs)
#   att = softmax_flat( q @ (k + P)^T )                    (P = pos_code, same
#   out = att * v                                           for all channels)
#
# Design (data-parallel over batch, 4 per core):
#   * P and biases folded into the projections via an augmented x
#     (ones-row + P-row), so K+P comes straight out of PSUM.
#   * Q,K projected with x-chunks stationary -> tiles come out [w, h, ch];
#     per-channel att matmuls then read [w, 128] slices.
#   * Softmax without max-subtraction (logits bounded ~|45|, fp32 exp,
#     bf16 E storage).
#   * The softmax reciprocal 1/s_c is folded into the V projection
#     WEIGHTS (Wv is [66,64]): V is projected only after the denominator
#     is known, so V comes out of the PE already scaled (wvs must be
#     bf16: sinv can be ~e^-45 which underflows fp16).
#   * All PSUM->SBUF evictions use contiguous destination APs (strided
#     writes measured 5-6 ns/elem) and are split ScalarE/VectorE by knobs.
#   * Final multiply is grouped by H (not C): each group depends on one
#     V-eviction group only, so it pipelines right behind the V
#     projection with no tail.  The E-operand is read through a strided
#     view; groups are split between GpSimd (otherwise idle) and VectorE.
#   * att matmuls interleave into the next batch's QK-projection stream
#     so the in-order PE/Act queues never head-of-line block.
#
# Layouts (per core):
#   xa   DRAM in  [4, 66, 16384] fp16   rows 0..63 = x, row 64 = 1.0 (bias),
#                                       row 65 = P.flatten() (K-only via waug)
#   waug DRAM in  [66, 192] fp16        cols 0:64 Wq^T | 64:128 Wk^T | 128:192 Wv^T
#   out  DRAM out [4, 128(w), 128(h), 64(c)] bf16  (host transposes to [b,c,h,w])

import sys

if "/opt/trn_rl_repo" not in sys.path:
    sys.path.insert(0, "/opt/trn_rl_repo")

import numpy as np
from contextlib import ExitStack

import concourse.bass as bass
import concourse.tile as tile
from concourse import bacc, mybir
from concourse.bass_utils import run_bass_kernel_spmd

N_CORES = 8
B_LOC = 4            # 32 batches / 8 cores
C = 64               # out channels
F = 128              # feature map size
S = F * F            # 16384 positions

FP16 = mybir.dt.float16
BF16 = mybir.dt.bfloat16
F32 = mybir.dt.float32

_BUILT = {}

# Engine split knobs: which group indices go to ScalarE (rest: VectorE).
QK_ON_ACT = set(range(16)) - {2, 5, 8, 11, 14}      # 11 of 16
V_ON_ACT = set(range(16)) - {1, 4, 7, 10, 13, 15}   # 10 of 16
# pass-B h-groups on GpSimd (rest: VectorE, same strided E read).
PASSB_ON_GPSIMD = {0, 1, 2, 4, 5, 6}                # 6 of 8


def _build_bass():
    nc = bacc.Bacc("TRN2", target_bir_lowering=False, debug=False)

    xa = nc.declare_dram_parameter("xa", [B_LOC, 66, S], FP16, isOutput=False)
    waug = nc.declare_dram_parameter("waug", [66, 192], FP16, isOutput=False)
    out = nc.declare_dram_parameter("out", [B_LOC, F, F, C], BF16, isOutput=True)

    with ExitStack() as ctx:
        tc = ctx.enter_context(tile.TileContext(nc))

        const = ctx.enter_context(tc.tile_pool(name="const", bufs=1))
        xpool = ctx.enter_context(tc.tile_pool(name="xpool", bufs=2))
        qkpool = ctx.enter_context(tc.tile_pool(name="qkpool", bufs=2))
        epool = ctx.enter_context(tc.tile_pool(name="epool", bufs=2))
        vpool = ctx.enter_context(tc.tile_pool(name="vpool", bufs=2))
        rpool = ctx.enter_context(tc.tile_pool(name="rpool", bufs=2))
        opool = ctx.enter_context(tc.tile_pool(name="opool", bufs=4))
        ps = ctx.enter_context(tc.tile_pool(name="ps", bufs=3, space="PSUM"))
        psv = ctx.enter_context(tc.tile_pool(name="psv", bufs=2, space="PSUM"))

        waug_sb = const.tile([66, 192], FP16, tag="waug")
        nc.sync.dma_start(out=waug_sb[:], in_=waug[:, :])
        ones_sb = const.tile([128, 128], BF16, tag="ones")
        nc.gpsimd.memset(ones_sb[:], 1.0)

        st = {}   # per-batch pipeline state

        def emit_xload(b):
            x_t = xpool.tile([66, S], FP16, tag="xt", name=f"xt_{b}")
            for xc in range(8):
                nc.sync.dma_start(out=x_t[:, xc * 2048:(xc + 1) * 2048],
                                  in_=xa[b, :, xc * 2048:(xc + 1) * 2048])
            st[b] = {"x": x_t}

        def emit_qk_group(b, g):
            s = st[b]
            if g == 0:
                s["qk"] = qkpool.tile([128, F, 128], FP16, tag="qk",
                                      name=f"qk_{b}")  # [w, h, q|k]
            pqk = ps.tile([128, 8, 128], F32, tag="ps", name=f"pqk_{b}_{g}")
            for jj in range(8):
                j = g * 8 + jj
                nc.tensor.matmul(
                    pqk[:, jj, :],
                    lhsT=s["x"][:, j * F:(j + 1) * F],
                    rhs=waug_sb[:, 0:128],
                    start=True, stop=True,
                )
            if g in QK_ON_ACT:
                nc.scalar.copy(s["qk"][:, g * 8:(g + 1) * 8, :], pqk[:, :, :])
            else:
                nc.vector.tensor_copy(s["qk"][:, g * 8:(g + 1) * 8, :],
                                      pqk[:, :, :])

        def emit_att_group(b, cg):
            s = st[b]
            if cg == 0:
                s["e"] = epool.tile([128, C, F], BF16, tag="e", name=f"e_{b}")
                s["r"] = rpool.tile([128, C], BF16, tag="r", name=f"r_{b}")
            c0 = cg * 8
            at = ps.tile([128, 8, 128], F32, tag="ps", name=f"at_{b}_{cg}")
            for cc in range(8):
                c = c0 + cc
                nc.tensor.matmul(
                    at[:, cc, :],
                    lhsT=s["qk"][:, :, 64 + c],   # (K+P)^T tile [w, v]
                    rhs=s["qk"][:, :, c],         # Q^T tile [w, h]
                    start=True, stop=True,
                )
            nc.scalar.activation(
                s["e"][:, c0:c0 + 8, :], at[:, :, :],
                mybir.ActivationFunctionType.Exp,
            )
            with nc.allow_low_precision("bf16 softmax denominators"):
                nc.vector.tensor_reduce(
                    s["r"][:, c0:c0 + 8], s["e"][:, c0:c0 + 8, :],
                    axis=mybir.AxisListType.X, op=mybir.AluOpType.add,
                )

        def emit_sinv(b):
            s = st[b]
            spt = ps.tile([128, 8, 128], F32, tag="ps", name=f"sp_{b}")
            sp = spt[:, 0, 0:64]
            nc.tensor.matmul(sp, lhsT=ones_sb[:], rhs=s["r"][:, :],
                             start=True, stop=True)
            sinv = rpool.tile([128, C], F32, tag="sinv", name=f"sinv_{b}")
            nc.vector.reciprocal(sinv[:, :], sp)
            wvs = rpool.tile([66, C], BF16, tag="wvs", name=f"wvs_{b}")
            nc.vector.tensor_mul(wvs[:, :], waug_sb[0:66, 128:192],
                                 sinv[0:66, :])
            s["wvs"] = wvs

        def emit_v_group(b, vg):
            s = st[b]
            if vg == 0:
                s["v"] = vpool.tile([128, F, C], BF16, tag="v",
                                    name=f"v_{b}")  # [w, h, c] natural layout
            pv = psv.tile([128, 8, 64], F32, tag="psv", name=f"pv_{b}_{vg}")
            for jj in range(8):
                j = vg * 8 + jj
                nc.tensor.matmul(
                    pv[:, jj, :],
                    lhsT=s["x"][:, j * F:(j + 1) * F],
                    rhs=s["wvs"][:, :],
                    start=True, stop=True,
                )
            dst = s["v"][:, vg * 8:(vg + 1) * 8, :]
            if vg in V_ON_ACT:
                nc.scalar.copy(dst, pv[:, :, :])
            else:
                nc.vector.tensor_copy(dst, pv[:, :, :])

        def emit_pass2(b, hg):
            s = st[b]
            h0 = hg * 16
            ot = opool.tile([128, 16, C], BF16, tag="ot", name=f"ot_{b}_{hg}")
            # E read through a transposed view: [w, c, h-slice] -> [w, h, c]
            e_view = s["e"][:, :, h0:h0 + 16].transpose([0, 2, 1])
            eng = nc.gpsimd if hg in PASSB_ON_GPSIMD else nc.vector
            eng.tensor_mul(ot[:, :, :], e_view, s["v"][:, h0:h0 + 16, :])
            nc.sync.dma_start(out=out[b, :, h0:h0 + 16, :], in_=ot[:])

        # ---- software pipeline ----
        # iteration i: QKproj(i) with att(i-1) interleaved, then sinv(i-1),
        # then Vproj(i-1) with pass2(i-1) chasing group-by-group.
        for i in range(B_LOC + 1):
            p = i if i < B_LOC else None            # QK projection batch
            c = i - 1 if i >= 1 else None           # att + V + pass2 batch

            if p is not None:
                emit_xload(p)
            for g in range(16):
                if c is not None and g < 8:
                    emit_att_group(c, g)
                if p is not None:
                    emit_qk_group(p, g)
            if c is not None:
                emit_sinv(c)
                for vg in range(16):
                    emit_v_group(c, vg)
                    if vg % 2 == 1:
                        emit_pass2(c, vg // 2)

    nc.compile()
    return nc


def _get_built():
    if "nc" not in _BUILT:
        _BUILT["nc"] = _build_bass()
    return _BUILT["nc"]


def _prep_inputs(x, wq, bq, wk, bk, wv, bv, pos_code):
    x = np.asarray(x, np.float32)
    pos = np.asarray(pos_code, np.float32)[0]          # identical across channels
    waug = np.zeros([66, 192], np.float32)
    waug[0:64, 0:64] = np.asarray(wq, np.float32).T
    waug[0:64, 64:128] = np.asarray(wk, np.float32).T
    waug[0:64, 128:192] = np.asarray(wv, np.float32).T
    waug[64, 0:64] = np.asarray(bq, np.float32)
    waug[64, 64:128] = np.asarray(bk, np.float32)
    waug[64, 128:192] = np.asarray(bv, np.float32)
    waug[65, 64:128] = 1.0                             # P-row hits K channels only
    waug16 = waug.astype(np.float16)

    pflat16 = pos.reshape(-1).astype(np.float16)
    xf = x.reshape(x.shape[0], x.shape[1], S)
    in_maps = []
    for core in range(N_CORES):
        xs = xf[core * B_LOC:(core + 1) * B_LOC]
        xa = np.empty([B_LOC, 66, S], np.float16)
        xa[:, 0:64] = xs.astype(np.float16)
        xa[:, 64] = np.float16(1.0)
        xa[:, 65] = pflat16[None, :]
        in_maps.append({"xa": xa, "waug": waug16})
    return in_maps


LAST_RESULTS = None


def kernel(x, wq, bq, wk, bk, wv, bv, pos_code, _trace=False):
    global LAST_RESULTS
    in_maps = _prep_inputs(x, wq, bq, wk, bk, wv, bv, pos_code)
    nc = _get_built()
    res = run_bass_kernel_spmd(nc, in_maps, core_ids=list(range(N_CORES)),
                               trace=_trace)
    LAST_RESULTS = res
    outs = []
    for core in range(N_CORES):
        o = np.asarray(res.results[core]["out"])       # [4, w, h, c] bf16
        outs.append(np.transpose(o.astype(np.float32), (0, 3, 2, 1)))
    return np.concatenate(outs, axis=0)



# revision 2
# speedup vs baseline: 1.0633x; 1.0633x over previous
# ARFSA attention kernel for 8 TRN2 NeuronCores (Bass/Tile), v5.
#
# Reference computation (per batch b, channel c):
#   q = Wq x + bq ; k = Wk x + bk ; v = Wv x + bv          (1x1 convs)
#   att = softmax_flat( q @ (k + P)^T )                    (P = pos_code)
#   out = att * v
#
# v5 design (data-parallel over batch, 4 per core) — differences vs v4:
#   * V is projected in the SAME pass as Q/K (sharing the x-chunk
#     stationary), decoupling the PE stream from the softmax: sinv is
#     applied at the end on the DVE via a broadcast view, not via scaled
#     V weights.  The per-batch PE stream is then QKV-proj + att only,
#     with no sinv->Vproj serialization.
#   * The V matmul writes PSUM with strided columns (psV [128, 64c, 8h])
#     so the V eviction lands in a c-major SBUF tile [w, c, h] matching
#     the E layout: pass2 becomes two fully contiguous bf16
#     tensor_tensor ops that qualify for the DVE 2x_1P perf mode.
#   * E is evicted c-major ([w, c, h]) directly from the att PSUM
#     ([128, 8c, 128h]) -- contiguous activation; softmax reduce is then
#     an innermost-axis DVE reduce.
#   * All PSUM tiles are bank-aligned: psQK [128,8,128] (2 banks,
#     shared pool with att psum, 3 bufs), psV [128,64,8] (1 bank, 2
#     bufs).
#
# Layouts (per core):
#   xa   DRAM in  [4, 66, 16384] fp16   rows 0..63 = x, row 64 = 1.0 (bias),
#                                       row 65 = P.flatten() (K-only via waug)
#   waug DRAM in  [66, 192] fp16        cols 0:64 Wq^T | 64:128 Wk^T | 128:192 Wv^T
#   out  DRAM out [4, 128(w), 64(c), 128(h)] bf16  (host transposes to [b,c,h,w])

import sys

if "/opt/trn_rl_repo" not in sys.path:
    sys.path.insert(0, "/opt/trn_rl_repo")

import numpy as np
from contextlib import ExitStack

import concourse.bass as bass
import concourse.tile as tile
from concourse import bacc, mybir
from concourse.bass_utils import run_bass_kernel_spmd

N_CORES = 8
B_LOC = 4            # 32 batches / 8 cores
C = 64               # out channels
F = 128              # feature map size
S = F * F            # 16384 positions
HALF = S // 2        # x loaded in two halves

FP16 = mybir.dt.float16
BF16 = mybir.dt.bfloat16
F32 = mybir.dt.float32

_BUILT = {}

# Engine split knobs: which group indices run on ScalarE (rest: VectorE).
QK_EVICT_ON_ACT = set(range(16)) - {3, 8, 13}       # 13 of 16 on Act
V_EVICT_ON_ACT = set(range(16))                     # all on Act
# pass2 stage knobs: which of the 4 pass2 groups run stage1/2 on GpSimd.
PASS2A_ON_GPS = set()
PASS2B_ON_GPS = set()


def _build_bass():
    nc = bacc.Bacc("TRN2", target_bir_lowering=False, debug=False)

    xa = nc.declare_dram_parameter("xa", [B_LOC, 66, S], FP16, isOutput=False)
    waug = nc.declare_dram_parameter("waug", [66, 192], FP16, isOutput=False)
    out = nc.declare_dram_parameter("out", [B_LOC, F, C, F], BF16, isOutput=True)

    with ExitStack() as ctx:
        tc = ctx.enter_context(tile.TileContext(nc))

        const = ctx.enter_context(tc.tile_pool(name="const", bufs=1))
        xpool = ctx.enter_context(tc.tile_pool(name="xpool", bufs=3))
        qkpool = ctx.enter_context(tc.tile_pool(name="qkpool", bufs=2))
        vpool = ctx.enter_context(tc.tile_pool(name="vpool", bufs=2))
        epool = ctx.enter_context(tc.tile_pool(name="epool", bufs=2))
        rpool = ctx.enter_context(tc.tile_pool(name="rpool", bufs=2))
        tpool = ctx.enter_context(tc.tile_pool(name="tpool", bufs=2))
        opool = ctx.enter_context(tc.tile_pool(name="opool", bufs=3))
        ps = ctx.enter_context(tc.tile_pool(name="ps", bufs=3, space="PSUM"))
        psv = ctx.enter_context(tc.tile_pool(name="psv", bufs=2, space="PSUM"))

        waug_sb = const.tile([66, 192], FP16, tag="waug")
        nc.sync.dma_start(out=waug_sb[:], in_=waug[:, :])
        ones_sb = const.tile([128, 128], BF16, tag="ones")
        nc.gpsimd.memset(ones_sb[:], 1.0)

        st = {}   # per-batch pipeline state

        def emit_xload(b, half):
            x_t = xpool.tile([66, HALF], FP16, tag="xt", name=f"xt_{b}_{half}")
            for xc in range(4):
                nc.sync.dma_start(
                    out=x_t[:, xc * 2048:(xc + 1) * 2048],
                    in_=xa[b, :, half * HALF + xc * 2048:
                           half * HALF + (xc + 1) * 2048])
            st.setdefault(b, {})[f"x{half}"] = x_t

        def emit_qkv_group(b, g):
            # 8 chunks: QK matmul (N=128, contiguous out) + V matmul
            # (N=64, strided columns into psV [128, 64, 8]).
            s = st[b]
            if g == 0:
                s["qk"] = qkpool.tile([128, F, 128], FP16, tag="qk",
                                      name=f"qk_{b}")   # [w, h, q|kp]
                s["v"] = vpool.tile([128, C, F], FP16, tag="v",
                                    name=f"v_{b}")      # [w, c, h]
            x_t = s["x0"] if g < 8 else s["x1"]
            goff = (g % 8) * 8
            pqk = ps.tile([128, 8, 128], F32, tag="ps", name=f"pqk_{b}_{g}")
            pv = psv.tile([128, C, 8], F32, tag="psv", name=f"pv_{b}_{g}")
            for jj in range(8):
                xs = x_t[:, (goff + jj) * F:(goff + jj + 1) * F]
                nc.tensor.matmul(pqk[:, jj, :], lhsT=xs,
                                 rhs=waug_sb[:, 0:128], start=True, stop=True)
                nc.tensor.matmul(pv[:, :, jj], lhsT=xs,
                                 rhs=waug_sb[:, 128:192], start=True, stop=True)
            eng = nc.scalar.copy if g in QK_EVICT_ON_ACT else nc.vector.tensor_copy
            eng(s["qk"][:, g * 8:(g + 1) * 8, :], pqk[:, :, :])
            eng2 = nc.scalar.copy if g in V_EVICT_ON_ACT else nc.vector.tensor_copy
            eng2(s["v"][:, :, g * 8:(g + 1) * 8], pv[:, :, :])

        def emit_att_group(b, cg):
            # 8 channels: att matmuls (strided qk reads), exp (Act,
            # contiguous c-major E), reduce (DVE, innermost axis).
            s = st[b]
            if cg == 0:
                s["e"] = epool.tile([128, C, F], BF16, tag="e", name=f"e_{b}")
                s["r"] = rpool.tile([128, C], BF16, tag="r", name=f"r_{b}")
            c0 = cg * 8
            at = ps.tile([128, 8, 128], F32, tag="ps", name=f"at_{b}_{cg}")
            for cc in range(8):
                c = c0 + cc
                nc.tensor.matmul(
                    at[:, cc, :],
                    lhsT=s["qk"][:, :, 64 + c],   # (K+P)^T tile [w, v]
                    rhs=s["qk"][:, :, c],         # Q^T tile [w, h]
                    start=True, stop=True,
                )
            nc.scalar.activation(
                s["e"][:, c0:c0 + 8, :], at[:, :, :],
                mybir.ActivationFunctionType.Exp,
            )
            with nc.allow_low_precision("bf16 softmax denominators"):
                nc.vector.tensor_reduce(
                    s["r"][:, c0:c0 + 8], s["e"][:, c0:c0 + 8, :],
                    axis=mybir.AxisListType.X, op=mybir.AluOpType.add,
                )

        def emit_sinv(b):
            s = st[b]
            spt = ps.tile([128, 8, 128], F32, tag="ps", name=f"sp_{b}")
            sp = spt[:, 0, 0:64]
            nc.tensor.matmul(sp, lhsT=ones_sb[:], rhs=s["r"][:, :],
                             start=True, stop=True)
            sinv = rpool.tile([128, C], F32, tag="sinv", name=f"sinv_{b}")
            nc.vector.reciprocal(sinv[:, :], sp)
            svd = rpool.tile([128, C, 2], BF16, tag="svd", name=f"svd_{b}")
            nc.vector.tensor_copy(svd[:, :, 0], sinv[:, :])
            nc.vector.tensor_copy(svd[:, :, 1], sinv[:, :])
            s["svd"] = svd

        def emit_pass2(b, hg):
            # 16 channels per group: t = E*V (2x_1P), ot = t*sinv_bcast
            # (2x_1P via the duplicated-pair broadcast view).
            s = st[b]
            c0 = hg * 16
            tt = tpool.tile([128, 16, F], BF16, tag="tt", name=f"tt_{b}_{hg}")
            ot = opool.tile([128, 16, F], BF16, tag="ot", name=f"ot_{b}_{hg}")
            e_sl = s["e"][:, c0:c0 + 16, :]
            v_sl = s["v"][:, c0:c0 + 16, :]
            eng1 = nc.gpsimd if hg in PASS2A_ON_GPS else nc.vector
            eng1.tensor_mul(tt[:, :, :], e_sl, v_sl)
            sv = s["svd"][:, c0:c0 + 16, :].unsqueeze(2)
            sv = sv.broadcast_to([128, 16, 64, 2])
            t4 = tt[:, :, :].rearrange("p a (b c) -> p a b c", c=2)
            o4 = ot[:, :, :].rearrange("p a (b c) -> p a b c", c=2)
            eng2 = nc.gpsimd if hg in PASS2B_ON_GPS else nc.vector
            eng2.tensor_mul(o4, t4, sv)
            nc.sync.dma_start(out=out[b, :, c0:c0 + 16, :], in_=ot[:])

        # ---- software pipeline ----
        # iteration i: QKVproj(i) with att(i-1) interleaved, then
        # sinv(i-1) and pass2(i-1) trailing on the DVE.
        emit_xload(0, 0)
        emit_xload(0, 1)
        for i in range(B_LOC + 1):
            p = i if i < B_LOC else None            # QKV projection batch
            c = i - 1 if i >= 1 else None           # att + pass2 batch

            if p is not None and p + 1 < B_LOC:
                emit_xload(p + 1, 0)
            for g in range(16):
                if p is not None:
                    emit_qkv_group(p, g)
                    if g == 7 and p + 1 < B_LOC:
                        emit_xload(p + 1, 1)
                if c is not None and g % 2 == 1:
                    emit_att_group(c, g // 2)
            if c is not None:
                emit_sinv(c)
                for hg in range(4):
                    emit_pass2(c, hg)

    nc.compile()
    return nc


def _get_built():
    if "nc" not in _BUILT:
        _BUILT["nc"] = _build_bass()
    return _BUILT["nc"]


def _prep_inputs(x, wq, bq, wk, bk, wv, bv, pos_code):
    x = np.asarray(x, np.float32)
    pos = np.asarray(pos_code, np.float32)[0]          # identical across channels
    waug = np.zeros([66, 192], np.float32)
    waug[0:64, 0:64] = np.asarray(wq, np.float32).T
    waug[0:64, 64:128] = np.asarray(wk, np.float32).T
    waug[0:64, 128:192] = np.asarray(wv, np.float32).T
    waug[64, 0:64] = np.asarray(bq, np.float32)
    waug[64, 64:128] = np.asarray(bk, np.float32)
    waug[64, 128:192] = np.asarray(bv, np.float32)
    waug[65, 64:128] = 1.0                             # P-row hits K channels only
    waug16 = waug.astype(np.float16)

    pflat16 = pos.reshape(-1).astype(np.float16)
    xf = x.reshape(x.shape[0], x.shape[1], S)
    in_maps = []
    for core in range(N_CORES):
        xs = xf[core * B_LOC:(core + 1) * B_LOC]
        xa = np.empty([B_LOC, 66, S], np.float16)
        xa[:, 0:64] = xs.astype(np.float16)
        xa[:, 64] = np.float16(1.0)
        xa[:, 65] = pflat16[None, :]
        in_maps.append({"xa": xa, "waug": waug16})
    return in_maps


LAST_RESULTS = None


def kernel(x, wq, bq, wk, bk, wv, bv, pos_code, _trace=False):
    global LAST_RESULTS
    in_maps = _prep_inputs(x, wq, bq, wk, bk, wv, bv, pos_code)
    nc = _get_built()
    res = run_bass_kernel_spmd(nc, in_maps, core_ids=list(range(N_CORES)),
                               trace=_trace)
    LAST_RESULTS = res
    outs = []
    for core in range(N_CORES):
        o = np.asarray(res.results[core]["out"])       # [4, w, c, h] bf16
        outs.append(np.transpose(o.astype(np.float32), (0, 2, 3, 1)))
    return np.concatenate(outs, axis=0)
